# revision 4
# baseline (speedup 1.0000x reference)
"""ConvVAE2d (patch VAE) TRN2 kernel.

Data-parallel over batch across 8 NeuronCores; VAE weights replicated.
Per core: B_shard=32 images -> 2048 patches of dim 768.
Layout strategy: activations kept transposed [feature | patch] so every
weight matrix streams in natural [K, M] layout. Matmuls run in bf16 with
fp32 PSUM accumulation; unfold/fold handled by 3-dim gather/scatter DMAs
plus PE transposes.
"""

import numpy as np

# Full-problem shapes (hardcoded; must match the grading inputs)
B, C, H, W = 256, 3, 128, 128
K = 16
GH, GW = H // K, W // K           # 8 x 8
P = C * K * K                     # 768
HID = 2048
L = 256
NCORES = 8
BS = B // NCORES                  # 32 images per core

# Tiling
PAIRS = BS // 2                   # 16 image pairs per core (128 patches each)
PPC = 4                           # pairs per chunk
CHUNKS = PAIRS // PPC             # 4 chunks
NC_ = PPC * 128                   # 512 patches per chunk

_CACHE = {}


def _build():
    from concourse import bacc, mybir
    import concourse.tile as tile
    from concourse.masks import make_identity

    f32 = mybir.dt.float32
    bf16 = mybir.dt.bfloat16
    AF = mybir.ActivationFunctionType

    nc = bacc.Bacc("TRN2", target_bir_lowering=False, debug=False)

    x = nc.dram_tensor("x", [BS, C, H, W], f32, kind="ExternalInput")
    w_enc1 = nc.dram_tensor("w_enc1", [P, HID], f32, kind="ExternalInput")
    b_enc1 = nc.dram_tensor("b_enc1", [HID], f32, kind="ExternalInput")
    w_enc2 = nc.dram_tensor("w_enc2", [HID, 2 * L], f32, kind="ExternalInput")
    b_enc2 = nc.dram_tensor("b_enc2", [2 * L], f32, kind="ExternalInput")
    w_dec1 = nc.dram_tensor("w_dec1", [L, HID], f32, kind="ExternalInput")
    b_dec1 = nc.dram_tensor("b_dec1", [HID], f32, kind="ExternalInput")
    w_dec2 = nc.dram_tensor("w_dec2", [HID, P], f32, kind="ExternalInput")
    b_dec2 = nc.dram_tensor("b_dec2", [P], f32, kind="ExternalInput")
    recon = nc.dram_tensor("recon", [BS, C, H, W], f32, kind="ExternalOutput")
    mu_lv = nc.dram_tensor("mu_lv", [BS, 2 * L, GH, GW], f32, kind="ExternalOutput")

    # [b, c, gh, gw, i, j] views for gather/scatter (j contiguous both sides)
    xr = x.ap().rearrange("b c (gh i) (gw j) -> b c gh gw i j", i=K, j=K)
    rr = recon.ap().rearrange("b c (gh i) (gw j) -> b c gh gw i j", i=K, j=K)
    # [ch, lt, p, img, ghgw] view of mu_logvar output
    mo = mu_lv.ap().rearrange(
        "(ch img) (lt p) gh gw -> ch lt p img (gh gw)", img=2 * PPC, p=128
    )

    KT1, MT1 = P // 128, HID // 128      # 6, 16
    KT2, LT2 = HID // 128, 2 * L // 128  # 16, 4
    KT3, MT3 = L // 128, HID // 128      # 2, 16
    KT4, PT4 = HID // 128, P // 128      # 16, 6

    with tile.TileContext(nc) as tc:
        with tc.tile_pool(name="const", bufs=1) as cpool, \
             tc.tile_pool(name="io", bufs=6) as iopool, \
             tc.tile_pool(name="a2", bufs=2) as a2pool, \
             tc.tile_pool(name="a1", bufs=1) as a1pool, \
             tc.tile_pool(name="psmm", bufs=4, space="PSUM") as psmm, \
             tc.tile_pool(name="pstr", bufs=2, space="PSUM") as pstr:

            ident = cpool.tile([128, 128], f32, tag="ident", name="ident")
            make_identity(nc, ident[:, :])

            # ---- weights resident in SBUF as bf16 (SWDGE cast DMA) ----
            def load_w(dram, ktiles, mdim, tagp):
                ts = []
                for kt in range(ktiles):
                    t = cpool.tile([128, mdim], bf16, tag=f"{tagp}_{kt}", name=f"{tagp}_{kt}")
                    nc.gpsimd.dma_start(
                        out=t[:, :], in_=dram.ap()[kt * 128:(kt + 1) * 128, :]
                    )
                    ts.append(t)
                return ts

            w1 = load_w(w_enc1, KT1, HID, "w1")
            w2 = load_w(w_enc2, KT2, 2 * L, "w2")
            w3 = load_w(w_dec1, KT3, HID, "w3")
            w4 = load_w(w_dec2, KT4, P, "w4")

            def load_b(dram, n, tag):
                t = cpool.tile([128, n], f32, tag=tag, name=tag)
                nc.sync.dma_start(
                    out=t[:, :], in_=dram.ap().rearrange("(t p) -> p t", p=128)
                )
                return t

            b1 = load_b(b_enc1, MT1, "b1")
            b2 = load_b(b_enc2, LT2, "b2")
            b3 = load_b(b_dec1, MT3, "b3")
            b4 = load_b(b_dec2, PT4, "b4")

            for ch in range(CHUNKS):
                # ---- unfold: gather + PE transpose -> PT tiles (bf16) ----
                pt = [a2pool.tile([128, NC_], bf16, tag=f"pt{t}", name=f"pt{t}") for t in range(KT1)]
                for pr in range(PPC):
                    gp = ch * PPC + pr          # global pair
                    for c in range(C):
                        x2 = iopool.tile([128, K * K], f32, tag="x2", name="x2")
                        x2r = x2[:, :].rearrange(
                            "(b2 gh gw) (i j) -> b2 gh gw i j",
                            b2=2, gh=GH, gw=GW, i=K, j=K,
                        )
                        for ib in range(2):
                            bb = gp * 2 + ib
                            for gh in range(GH):
                                nc.sync.dma_start(
                                    out=x2r[ib, gh], in_=xr[bb, c, gh]
                                )
                        for ih in range(2):
                            ps = pstr.tile([128, 128], f32, tag="pstr", name="pstr")
                            nc.tensor.transpose(
                                ps[:, :], x2[:, ih * 128:(ih + 1) * 128],
                                ident[:, :],
                            )
                            nc.vector.tensor_copy(
                                pt[c * 2 + ih][:, pr * 128:(pr + 1) * 128],
                                ps[:, :],
                            )

                # ---- L1: he = relu(w1.T @ pt + b1)  [bf16] ----
                he = [a1pool.tile([128, NC_], bf16, tag=f"he{m}", name=f"he{m}") for m in range(MT1)]
                for mt in range(MT1):
                    ps = psmm.tile([128, NC_], f32, tag="psmm", name="psmm")
                    for kt in range(KT1):
                        nc.tensor.matmul(
                            ps[:, :], w1[kt][:, mt * 128:(mt + 1) * 128],
                            pt[kt][:, :], start=(kt == 0), stop=(kt == KT1 - 1),
                        )
                    nc.scalar.activation(
                        he[mt][:, :], ps[:, :], AF.Relu, bias=b1[:, mt:mt + 1]
                    )

                # ---- L2: lv = w2.T @ he + b2 (f32); z = cast(lv[:2]) ----
                lv = [a2pool.tile([128, NC_], f32, tag=f"lv{l}", name=f"lv{l}") for l in range(LT2)]
                z = [a2pool.tile([128, NC_], bf16, tag=f"z{l}", name=f"z{l}") for l in range(KT3)]
                for lt in range(LT2):
                    ps = psmm.tile([128, NC_], f32, tag="psmm", name="psmm")
                    for kt in range(KT2):
                        nc.tensor.matmul(
                            ps[:, :], w2[kt][:, lt * 128:(lt + 1) * 128],
                            he[kt][:, :], start=(kt == 0), stop=(kt == KT2 - 1),
                        )
                    nc.scalar.activation(
                        lv[lt][:, :], ps[:, :], AF.Identity, bias=b2[:, lt:lt + 1]
                    )
                    if lt < KT3:
                        nc.vector.tensor_copy(z[lt][:, :], lv[lt][:, :])

                # ---- L3: hd = relu(w3.T @ z + b3)  [bf16] ----
                hd = [a1pool.tile([128, NC_], bf16, tag=f"hd{m}", name=f"hd{m}") for m in range(MT3)]
                for mt in range(MT3):
                    ps = psmm.tile([128, NC_], f32, tag="psmm", name="psmm")
                    for kt in range(KT3):
                        nc.tensor.matmul(
                            ps[:, :], w3[kt][:, mt * 128:(mt + 1) * 128],
                            z[kt][:, :], start=(kt == 0), stop=(kt == KT3 - 1),
                        )
                    nc.scalar.activation(
                        hd[mt][:, :], ps[:, :], AF.Relu, bias=b3[:, mt:mt + 1]
                    )

                # ---- L4: dec = sigmoid(w4.T @ hd + b4)  [f32] ----
                dec = [a2pool.tile([128, NC_], f32, tag=f"dec{t}", name=f"dec{t}") for t in range(PT4)]
                for t4 in range(PT4):
                    ps = psmm.tile([128, NC_], f32, tag="psmm", name="psmm")
                    for kt in range(KT4):
                        nc.tensor.matmul(
                            ps[:, :], w4[kt][:, t4 * 128:(t4 + 1) * 128],
                            hd[kt][:, :], start=(kt == 0), stop=(kt == KT4 - 1),
                        )
                    nc.scalar.activation(
                        dec[t4][:, :], ps[:, :], AF.Sigmoid, bias=b4[:, t4:t4 + 1]
                    )

                # ---- mu_logvar out ----
                for lt in range(LT2):
                    nc.sync.dma_start(
                        out=mo[ch, lt],
                        in_=lv[lt][:, :].rearrange("p (img q) -> p img q", img=2 * PPC),
                    )

                # ---- fold: PE transpose back + scatter ----
                for pr in range(PPC):
                    gp = ch * PPC + pr
                    for c in range(C):
                        xo = iopool.tile([128, K * K], f32, tag="xo", name="xo")
                        for ih in range(2):
                            ps = pstr.tile([128, 128], f32, tag="pstr", name="pstr")
                            nc.tensor.transpose(
                                ps[:, :],
                                dec[c * 2 + ih][:, pr * 128:(pr + 1) * 128],
                                ident[:, :],
                            )
                            nc.vector.tensor_copy(
                                xo[:, ih * 128:(ih + 1) * 128], ps[:, :]
                            )
                        xor = xo[:, :].rearrange(
                            "(b2 gh gw) (i j) -> b2 gh gw i j",
                            b2=2, gh=GH, gw=GW, i=K, j=K,
                        )
                        for ib in range(2):
                            bb = gp * 2 + ib
                            for gh in range(GH):
                                nc.scalar.dma_start(
                                    out=rr[bb, c, gh], in_=xor[ib, gh]
                                )

    nc.compile()
    return nc


def _get_nc():
    if "nc" not in _CACHE:
        _CACHE["nc"] = _build()
    return _CACHE["nc"]


def kernel(x, w_enc1, b_enc1, w_enc2, b_enc2, w_dec1, b_dec1, w_dec2, b_dec2):
    from concourse.bass_utils import run_bass_kernel_spmd

    nc = _get_nc()
    x = np.ascontiguousarray(np.asarray(x, dtype=np.float32))
    shared = {
        "w_enc1": np.ascontiguousarray(np.asarray(w_enc1, np.float32)),
        "b_enc1": np.ascontiguousarray(np.asarray(b_enc1, np.float32)),
        "w_enc2": np.ascontiguousarray(np.asarray(w_enc2, np.float32)),
        "b_enc2": np.ascontiguousarray(np.asarray(b_enc2, np.float32)),
        "w_dec1": np.ascontiguousarray(np.asarray(w_dec1, np.float32)),
        "b_dec1": np.ascontiguousarray(np.asarray(b_dec1, np.float32)),
        "w_dec2": np.ascontiguousarray(np.asarray(w_dec2, np.float32)),
        "b_dec2": np.ascontiguousarray(np.asarray(b_dec2, np.float32)),
    }
    in_maps = [
        {"x": x[i * BS:(i + 1) * BS], **shared} for i in range(NCORES)
    ]
    res = run_bass_kernel_spmd(nc, in_maps, core_ids=list(range(NCORES)))
    recon = np.concatenate([res.results[i]["recon"] for i in range(NCORES)], axis=0)
    mlv = np.concatenate([res.results[i]["mu_lv"] for i in range(NCORES)], axis=0)
    return recon, mlv
